# revision 1
# baseline (speedup 1.0000x reference)
"""Trainium2 Bass kernel for Gumbel 2:4-masked Linear (tensor-parallel over out_features).

Math (matches the reference in forward value):
  idx    = first-occurrence argmax over 6 logits per 4-weight block,
           logits = choice_weights + gumbel_noise (choice constant -> scalar add)
  mask   = MASKING_PATTERNS[idx]          (six 2-of-4 binary patterns)
  out    = x @ (weight * mask).T + bias

Distribution: 8 NeuronCores, sharded by output rows (512 rows/core). Mask
generation and the masked GEMM are fully local; outputs concatenated on host.
x is transposed once on the host (xT [K, T]) so the GEMM's stationary operand
streams straight from DRAM with no on-chip transposes.

On-core pipeline (SPMD, per core):
  phase 1 (k-chunked): gumbel tile -> per-block max (DVE) -> exact first-max
      one-hot via prefix products of (l_p < m) compares -> mask columns by
      telescoping sums (DVE+GPSIMD, bf16 temps) -> masked weight ->
      PE-transpose into resident WmT [k, o] (float32r).
  phase 2: stream xT strips [k, 128 t] from DRAM, float32r matmuls accumulate
      psum [128 t, 512 o] over 32 k-tiles, bias add, DMA out.
"""

import numpy as np

N_CORES = 8
T = 4096          # tokens = 2*2048
K = 4096          # in_features
O_FULL = 4096     # out_features
O = O_FULL // N_CORES          # 512 out rows per core
GUM_COLS = K // 4 * 6          # 6144 logit floats per weight row
N_KC = 4                       # k chunks in phase 1
KC_K = K // N_KC               # 1024 k per chunk
KC_B = KC_K // 4               # 256 blocks per chunk row
KC_G = KC_B * 6                # 1536 logit floats per chunk row
N_KT = K // 128                # 32 k-tiles for the GEMM
N_OT = O // 128                # 4 o-tiles per core
N_TT = T // 128                # 32 token strips

_prog_cache = {}


def _build_program(mode, const_c, repeats=1):
    """mode: 'const' (choice folded to scalar) or 'full' (add choice tensor)."""
    import concourse.bacc as bacc
    import concourse.bass as bass
    import concourse.mybir as mybir
    import concourse.tile as tile
    from concourse.masks import make_identity

    f32 = mybir.dt.float32
    f32r = mybir.dt.float32r
    bf16 = mybir.dt.bfloat16
    Alu = mybir.AluOpType

    nc = bacc.Bacc(trn_type="TRN2")
    xt_d = nc.declare_dram_parameter("xt", [K, T], f32r, isOutput=False)
    w_d = nc.declare_dram_parameter("w", [O, K], f32, isOutput=False)
    b_d = nc.declare_dram_parameter("b", [1, O], f32, isOutput=False)
    g_d = nc.declare_dram_parameter("g", [O, GUM_COLS], f32, isOutput=False)
    if mode == "full":
        cw_d = nc.declare_dram_parameter("cw", [O, GUM_COLS], f32, isOutput=False)
    out_d = nc.declare_dram_parameter("out", [T, O], f32, isOutput=True)
    # [K, T] viewed as [kp=128, kt=32, t] for per-strip loads
    xt_v = xt_d.rearrange("(a p) t -> p a t", p=128)

    with tile.TileContext(nc) as tc:
        with (
            tc.tile_pool(name="singles", bufs=1) as singles,
            tc.tile_pool(name="wmt", bufs=N_KC) as wmt_pool,
            tc.tile_pool(name="gum", bufs=2) as gum_pool,
            tc.tile_pool(name="wtile", bufs=2) as w_pool,
            tc.tile_pool(name="mtmp", bufs=2) as mtmp,
            tc.tile_pool(name="xt", bufs=4) as xt_pool,
            tc.tile_pool(name="outs", bufs=2) as out_pool,
            tc.tile_pool(name="ps_xpose", bufs=1, space="PSUM") as ps_xpose,
            tc.tile_pool(name="ps_gemm", bufs=6, space="PSUM") as ps_gemm,
        ):
            ident_f32 = singles.tile([128, 128], f32)
            make_identity(nc, ident_f32)
            ident = singles.tile([128, 128], f32r, name="ident_r")
            nc.scalar.copy(ident, ident_f32)
            bias_s = singles.tile([128, O], f32)
            nc.gpsimd.dma_start(
                out=bias_s,
                in_=bass.AP(tensor=b_d, offset=0, ap=[[0, 128], [1, O]]),
            )

            # resident transposed masked weight, one tile per k chunk:
            # wmt[kc][p=k%128, j=kt within chunk, o]
            wmt = [
                wmt_pool.tile([128, N_KC * 2, O], f32r, name=f"wmt{i}", tag=f"wmt{i}", bufs=1)
                for i in range(N_KC)
            ]

            for _rep in range(repeats):
                # ------------- phase 1: mask + masked weight + transpose ----
                for kc in range(N_KC):
                    for ot in range(N_OT):
                        rows = slice(ot * 128, (ot + 1) * 128)
                        g_t = gum_pool.tile([128, KC_G], f32, tag="gum")
                        nc.sync.dma_start(
                            out=g_t, in_=g_d[rows, kc * KC_G:(kc + 1) * KC_G]
                        )
                        if mode == "full":
                            cw_t = gum_pool.tile([128, KC_G], f32, tag="cw")
                            nc.sync.dma_start(
                                out=cw_t, in_=cw_d[rows, kc * KC_G:(kc + 1) * KC_G]
                            )
                            nc.vector.tensor_add(g_t, g_t, cw_t)
                        elif const_c != 0.0:
                            nc.vector.tensor_scalar_add(g_t, g_t, const_c)
                        g3 = g_t.rearrange("p (b s) -> p b s", s=6)

                        m = mtmp.tile([128, KC_B], f32, tag="m")
                        nc.vector.tensor_reduce(
                            m, g3, axis=mybir.AxisListType.X, op=Alu.max
                        )

                        # exact first-max one-hot via prefix products
                        # c1=f0, c_{p+1}=c_p*f_p with f_p=(l_p<m); e0=(l0>=m)
                        e0 = mtmp.tile([128, KC_B], bf16, tag="e0")
                        c1 = mtmp.tile([128, KC_B], bf16, tag="c1")
                        nc.vector.tensor_tensor(c1, g3[:, :, 0], m, op=Alu.is_lt)
                        nc.vector.tensor_tensor(e0, g3[:, :, 0], m, op=Alu.is_ge)
                        f1 = mtmp.tile([128, KC_B], bf16, tag="f1")
                        f2 = mtmp.tile([128, KC_B], bf16, tag="f2")
                        f3 = mtmp.tile([128, KC_B], bf16, tag="f3")
                        f4 = mtmp.tile([128, KC_B], bf16, tag="f4")
                        nc.vector.tensor_tensor(f1, g3[:, :, 1], m, op=Alu.is_lt)
                        nc.vector.tensor_tensor(f2, g3[:, :, 2], m, op=Alu.is_lt)
                        nc.vector.tensor_tensor(f3, g3[:, :, 3], m, op=Alu.is_lt)
                        nc.vector.tensor_tensor(f4, g3[:, :, 4], m, op=Alu.is_lt)
                        c2 = mtmp.tile([128, KC_B], bf16, tag="c2")
                        c3 = mtmp.tile([128, KC_B], bf16, tag="c3")
                        c4 = mtmp.tile([128, KC_B], bf16, tag="c4")
                        c5 = mtmp.tile([128, KC_B], bf16, tag="c5")
                        nc.gpsimd.tensor_mul(c2, c1, f1)
                        nc.gpsimd.tensor_mul(c3, c2, f2)
                        nc.gpsimd.tensor_mul(c4, c3, f3)
                        nc.gpsimd.tensor_mul(c5, c4, f4)

                        w_t = w_pool.tile([128, KC_K], f32, tag="w")
                        nc.sync.dma_start(
                            out=w_t, in_=w_d[rows, kc * KC_K:(kc + 1) * KC_K]
                        )
                        w4 = w_t.rearrange("p (b s) -> p b s", s=4)
                        wm = w_pool.tile([128, KC_K], f32r, tag="wm")
                        wm4 = wm.rearrange("p (b s) -> p b s", s=4)

                        t0 = mtmp.tile([128, KC_B], bf16, tag="t0")
                        t1 = mtmp.tile([128, KC_B], bf16, tag="t1")
                        t2 = mtmp.tile([128, KC_B], bf16, tag="t2")
                        # col0 = c3
                        nc.vector.tensor_mul(wm4[:, :, 0], w4[:, :, 0], c3)
                        # col1 = c1 - c3 + c5
                        nc.gpsimd.tensor_sub(t0, c1, c3)
                        nc.gpsimd.tensor_add(t0, t0, c5)
                        nc.vector.tensor_mul(wm4[:, :, 1], w4[:, :, 1], t0)
                        # col2 = e0 + (c2-c3) + (c4-c5)
                        nc.gpsimd.tensor_sub(t1, c2, c3)
                        nc.gpsimd.tensor_sub(t2, c4, c5)
                        nc.gpsimd.tensor_add(t1, t1, t2)
                        nc.gpsimd.tensor_add(t1, t1, e0)
                        nc.vector.tensor_mul(wm4[:, :, 2], w4[:, :, 2], t1)
                        # col3 = e0 + (c1-c2) + (c3-c4)
                        nc.vector.tensor_sub(t2, c1, c2)
                        nc.vector.tensor_sub(t0, c3, c4)
                        nc.vector.tensor_add(t2, t2, t0)
                        nc.vector.tensor_add(t2, t2, e0)
                        nc.vector.tensor_mul(wm4[:, :, 3], w4[:, :, 3], t2)

                        # transpose 8 [128,128] subtiles -> wmt[kc][:, :, ot*128:]
                        ps = ps_xpose.tile([128, 1024], f32r, tag="psx")
                        for j in range(8):
                            nc.tensor.transpose(
                                ps[:, j * 128:(j + 1) * 128],
                                wm[:, j * 128:(j + 1) * 128],
                                ident,
                            )
                        nc.scalar.copy(
                            wmt[kc][:, :, ot * 128:(ot + 1) * 128],
                            ps.rearrange("p (a b) -> p a b", a=8),
                        )

                # ------------- phase 2: stream xT, GEMM --------------------
                for tt in range(N_TT):
                    trows = slice(tt * 128, (tt + 1) * 128)
                    xt = xt_pool.tile([128, N_KT, 128], f32r, tag="xt")
                    nc.sync.dma_start(out=xt, in_=xt_v[:, :, trows])

                    acc = ps_gemm.tile([128, O], f32, tag="acc")
                    for kc in range(N_KC):
                        for j in range(N_KC * 2):
                            kt = kc * (N_KC * 2) + j
                            nc.tensor.matmul(
                                acc,
                                xt[:, kt, :],
                                wmt[kc][:, j, :],
                                start=(kt == 0),
                                stop=(kt == N_KT - 1),
                            )
                    o_t = out_pool.tile([128, O], f32, tag="o")
                    nc.vector.tensor_add(o_t, acc, bias_s)
                    nc.sync.dma_start(out=out_d[trows, :], in_=o_t)

    nc.compile()
    return nc


def _get_program(mode, const_c):
    key = (mode, const_c)
    if key not in _prog_cache:
        _prog_cache[key] = _build_program(mode, const_c)
    return _prog_cache[key]


def kernel(x, weight, bias, choice_weights, gumbel_noise):
    from concourse.bass_utils import run_bass_kernel_spmd

    x = np.asarray(x, dtype=np.float32).reshape(T, K)
    xt = np.ascontiguousarray(x.T)
    w = np.ascontiguousarray(np.asarray(weight, dtype=np.float32))
    b = np.ascontiguousarray(np.asarray(bias, dtype=np.float32)).reshape(1, O_FULL)
    cw = np.asarray(choice_weights, dtype=np.float32)
    g = np.asarray(gumbel_noise, dtype=np.float32).reshape(O_FULL, GUM_COLS)

    c0 = float(cw.flat[0])
    is_const = bool((cw == c0).all())
    mode = "const" if is_const else "full"
    nc = _get_program(mode, c0 if is_const else None)

    in_maps = []
    for c in range(N_CORES):
        rows = slice(c * O, (c + 1) * O)
        m = {
            "xt": xt,
            "w": np.ascontiguousarray(w[rows]),
            "b": np.ascontiguousarray(b[:, rows]),
            "g": np.ascontiguousarray(g[rows]),
        }
        if mode == "full":
            m["cw"] = np.ascontiguousarray(
                cw.reshape(O_FULL, GUM_COLS)[rows]
            )
        in_maps.append(m)

    res = run_bass_kernel_spmd(nc, in_maps, list(range(N_CORES)))
    parts = [res.results[c]["out"] for c in range(N_CORES)]
    out = np.concatenate(parts, axis=1)  # [T, O_FULL]
    return out.reshape(2, 2048, O_FULL)



# revision 3
# speedup vs baseline: 1.3597x; 1.3597x over previous
"""Trainium2 Bass kernel for Gumbel 2:4-masked Linear (tensor-parallel over out_features).

Forward value (matches reference): mask = PATTERNS[argmax(cw + g, axis=-1)],
out = x @ (W * mask).T + b.  With constant choice_weights the argmax is
shift-invariant, so the mask is PATTERNS[argmax(g)].

v2 design (cost-model driven):
  - bf16 GEMM (x, W, masked W): same PE rate as f32r in the cost model but
    half the DMA traffic and 2-4x DVE rate on mask math. Gumbel stays f32
    (bf16 would flip ~1e-3 of the argmaxes).
  - Mask build per (kc, ot) tile [128 o, 256 blocks]: one max-reduce, one
    batched is_ge (6 planes at once, broadcast max), 5 batched/single bf16
    adds to form the 4 mask columns, one bf16 mul with host-plane-packed W.
  - Reduce+compare engine alternates DVE/Pool per iter to balance; adds+mul
    on DVE; PSUM->SBUF copies on Act (otherwise idle); transposes on PE.
  - Bias folded into each PSUM accumulation chain as a rank-1 matmul
    (ones[1,128].T @ bias[1,512]); Act copies PSUM->SBUF bf16; DMA out bf16.
  - Host pre-packs xT/w so every DMA moves >=512B contiguous runs
    (full 360 GB/s in the model): per-strip xT descriptors are 8 KB.
  - xt strip loads interleaved into phase 1 at chunk boundaries so the GEMM
    can start as soon as wmt[0] lands.
"""

import numpy as np

N_CORES = 8
T = 4096            # tokens
K = 4096            # in_features
O_FULL = 4096
O = O_FULL // N_CORES        # 512 out rows per core
B = K // 4                   # 1024 blocks per out row
GUM_COLS = B * 6             # 6144
N_KC = 4                     # k chunks
KC_B = B // N_KC             # 256 blocks per chunk
N_OT = O // 128              # 4 o-tiles
N_STRIP = 16                 # token strips
TS = T // N_STRIP            # 256 tokens per strip

import os as _os

# iters whose reduce+compare run on Pool (rest on DVE); tuned for balance
_POOL_SETS = {
    "16": list(range(16)),
    "14": [i for i in range(16) if i not in (0, 3)],
    "12": [1, 2, 4, 5, 6, 8, 9, 11, 12, 13, 14, 15],
    "10": [1, 2, 5, 6, 8, 9, 11, 12, 14, 15],
    "8": [1, 3, 5, 7, 8, 10, 12, 14],
}
POOL_ITERS = frozenset(_POOL_SETS[_os.environ.get("KV2_POOL", "10")])
# strips using bias-matmul zeroing + o-sliced chunk-0 + Act-copy out
N_EARLY = int(_os.environ.get("KV2_EARLY", "0"))
# xt (strip, half) prefetches at each chunk boundary
_XT_PLANS = {
    "A": [[(0, 0), (1, 0)], [(2, 0), (0, 1)], [(1, 1), (3, 0), (2, 1)],
          [(3, 1)]],
    "B": [[(0, 0)], [(1, 0)], [(0, 1), (2, 0)], [(1, 1), (3, 0)]],
    "C": [[(0, 0)], [(1, 0), (2, 0)], [(0, 1), (1, 1)], [(2, 1), (3, 0)]],
    "D": [[(0, 0)], [(1, 0)], [(2, 0)], [(0, 1), (3, 0)]],
    "E": [[(0, 0)], [(1, 0), (2, 0)], [(3, 0), (0, 1), (4, 0)],
          [(1, 1), (5, 0)]],
    "F": [[(0, 0)], [(1, 0), (2, 0)], [(3, 0), (4, 0), (0, 1)],
          [(5, 0), (1, 1), (6, 0), (2, 1)]],
    "G": [[(0, 0), (1, 0)], [(2, 0), (3, 0)], [(4, 0), (0, 1), (5, 0)],
          [(1, 1), (6, 0), (2, 1)]],
}
XT_PLAN = _XT_PLANS[_os.environ.get("KV2_XT", "C")]

_prog_cache = {}


def _build_program(mode):
    """mode: 'const' (choice folded away) or 'full' (adds choice tensor)."""
    import concourse.bacc as bacc
    import concourse.bass as bass
    import concourse.mybir as mybir
    import concourse.tile as tile
    from concourse.masks import make_identity

    f32 = mybir.dt.float32
    bf16 = mybir.dt.bfloat16
    Alu = mybir.AluOpType

    nc = bacc.Bacc(trn_type="TRN2")
    # xt packed: [strip, kbp, (kc, s, kbt), t] -> [16*128, 32*256] bf16
    xt_d = nc.declare_dram_parameter("xt", [N_STRIP * 128, 32 * TS], bf16,
                                     isOutput=False)
    # w packed: [(kc, ot), po, (s, kb)] -> [16*128, 1024] bf16
    w_d = nc.declare_dram_parameter("w", [16 * 128, 1024], bf16, isOutput=False)
    b_d = nc.declare_dram_parameter("b", [1, O], f32, isOutput=False)
    g_d = nc.declare_dram_parameter("g", [O, GUM_COLS], f32, isOutput=False)
    if mode == "full":
        cw_d = nc.declare_dram_parameter("cw", [O, GUM_COLS], f32, isOutput=False)
    out_d = nc.declare_dram_parameter("out", [T, O], bf16, isOutput=True)

    with tile.TileContext(nc) as tc:
        with (
            tc.tile_pool(name="singles", bufs=1) as singles,
            tc.tile_pool(name="wmt", bufs=N_KC) as wmt_pool,
            tc.tile_pool(name="gum", bufs=4) as gum_pool,
            tc.tile_pool(name="wtile", bufs=4) as w_pool,
            tc.tile_pool(name="msk", bufs=3) as msk,
            tc.tile_pool(name="xth", bufs=8) as xt_pool,
            tc.tile_pool(name="outs", bufs=3) as out_pool,
            tc.tile_pool(name="ps_xpose", bufs=int(_os.environ.get("KV2_XPB", "1")),
                         space="PSUM") as ps_xpose,
            tc.tile_pool(name="ps_gemm", bufs=int(_os.environ.get("KV2_ACCB", "7")),
                         space="PSUM") as ps_gemm,
        ):
            ident_f32 = singles.tile([128, 128], f32)
            make_identity(nc, ident_f32)
            ident = singles.tile([128, 128], bf16, name="ident_bf")
            nc.vector.tensor_copy(ident, ident_f32)
            bias_s = singles.tile([128, O], f32, name="bias_s")
            nc.gpsimd.dma_start(
                out=bias_s,
                in_=bass.AP(tensor=b_d, offset=0, ap=[[0, 128], [1, O]]),
            )
            bias_bf = singles.tile([1, O], bf16, name="bias_bf")
            nc.vector.tensor_copy(bias_bf, bias_s[0:1, :])
            ones_t = singles.tile([1, 128], bf16, name="ones")
            nc.vector.memset(ones_t, 1.0)

            # resident transposed masked weight, one tile per k chunk:
            # wmt[kc][kbp, j=(s,kbt), o]
            wmt = [
                wmt_pool.tile([128, 8, O], bf16, name=f"wmt{i}", tag=f"wmt{i}",
                              bufs=1)
                for i in range(N_KC)
            ]

            xt_tiles = {}  # (strip, half) -> tile

            def issue_xt(s, h):
                if (s, h) in xt_tiles:
                    return
                t_ = xt_pool.tile([128, 16, TS], bf16, tag="xth")
                nc.sync.dma_start(
                    out=t_,
                    in_=xt_d[s * 128:(s + 1) * 128,
                             h * 16 * TS:(h + 1) * 16 * TS],
                )
                xt_tiles[(s, h)] = t_

            # ---------------- phase 1: masks + masked W^T -----------------
            for kc in range(N_KC):
                g_tiles = [None] * N_OT
                w_tiles = [None] * N_OT

                def _load_g(ot, kc=kc, g_tiles=g_tiles):
                    rows = slice(ot * 128, (ot + 1) * 128)
                    t_ = gum_pool.tile([128, KC_B * 6], f32, tag="gum",
                                       name=f"g_{kc}_{ot}")
                    nc.sync.dma_start(
                        out=t_, in_=g_d[rows, kc * KC_B * 6:(kc + 1) * KC_B * 6]
                    )
                    g_tiles[ot] = t_

                def _load_w(ot, kc=kc, w_tiles=w_tiles):
                    t_ = w_pool.tile([128, 1024], bf16, tag="w",
                                     name=f"w_{kc}_{ot}")
                    nc.sync.dma_start(
                        out=t_, in_=w_d[(kc * N_OT + ot) * 128:
                                        (kc * N_OT + ot + 1) * 128, :]
                    )
                    w_tiles[ot] = t_

                if kc == 0:
                    # cold start: g00 first (mask pipe), then all w (muls),
                    # then remaining g
                    _load_g(0)
                    for ot in range(N_OT):
                        _load_w(ot)
                    for ot in range(1, N_OT):
                        _load_g(ot)
                else:
                    for ot in range(N_OT):
                        _load_g(ot)
                    for ot in range(N_OT):
                        _load_w(ot)
                for ot in range(N_OT):
                    it = kc * N_OT + ot
                    # Pool is add/sub/mult-only on TRN2; reduce+compare are
                    # DVE-only. Pool takes adds+mul for POOL_ITERS.
                    eng = nc.gpsimd if it in POOL_ITERS else nc.vector
                    rows = slice(ot * 128, (ot + 1) * 128)
                    g_t = g_tiles[ot]
                    if mode == "full":
                        cw_t = gum_pool.tile([128, KC_B * 6], f32, tag="cw")
                        nc.sync.dma_start(
                            out=cw_t,
                            in_=cw_d[rows, kc * KC_B * 6:(kc + 1) * KC_B * 6],
                        )
                        nc.vector.tensor_add(g_t, g_t, cw_t)
                    g3 = g_t.rearrange("p (b s) -> p b s", s=6)

                    m = msk.tile([128, KC_B], f32, tag="m")
                    nc.vector.tensor_reduce(m, g3, axis=mybir.AxisListType.X,
                                            op=Alu.max)

                    # one-hot planes e[s][kb] (multi-hot only on exact ties)
                    e_t = msk.tile([128, 6, KC_B], bf16, tag="e")
                    gsb = g_t.rearrange("p (b s) -> p s b", s=6)
                    m_b = m.unsqueeze(1).broadcast_to([128, 6, KC_B])
                    nc.vector.tensor_tensor(e_t, gsb, m_b, op=Alu.is_ge)

                    # cols storage order: [col2, col1, col3, col0]
                    # col0=e3+e4+e5  col1=e1+e2+e5  col2=e0+e2+e4  col3=e0+e1+e3
                    ev = e_t  # [128, 6, 256]
                    s2 = msk.tile([128, 2, KC_B], bf16, tag="s2")
                    # s2 = [e0+e1, e4+e5]  (step-slice: DVE only)
                    nc.vector.tensor_add(s2, ev[:, 0::4, :], ev[:, 1::4, :])
                    t2 = msk.tile([128, 2, KC_B], bf16, tag="t2")
                    # t2 = [e0+e2, e1+e2]
                    eng.tensor_add(t2, ev[:, 0:2, :],
                                   ev[:, 2:3, :].broadcast_to([128, 2, KC_B]))
                    cols = msk.tile([128, 4, KC_B], bf16, tag="cols")
                    # [col2, col1] = t2 + [e4, e5]
                    eng.tensor_add(cols[:, 0:2, :], t2, ev[:, 4:6, :])
                    # [col3, col0] = s2 + e3
                    eng.tensor_add(cols[:, 2:4, :], s2,
                                   ev[:, 3:4, :].broadcast_to([128, 2, KC_B]))

                    w_t = w_tiles[ot]
                    # wm[o, s, kb] = w[o, s, kb] * col_s[o, kb]
                    # w packed with s-plane order (2, 1, 3, 0) to match cols
                    wm = w_pool.tile([128, 1024], bf16, tag="wm")
                    eng.tensor_mul(
                        wm, w_t, cols.rearrange("p s b -> p (s b)")
                    )

                    # transpose 8 subtiles [o, f] -> [f, o]; psum tile j holds
                    # f-range [j*128, (j+1)*128) (same map as host K_IDX)
                    ps = ps_xpose.tile([128, 8, 128], bf16, tag="psx")
                    wmv = wm.rearrange("p (j b) -> p j b", j=8)
                    for j in range(8):
                        nc.tensor.transpose(ps[:, j, :], wmv[:, j, :], ident)
                    nc.scalar.copy(
                        wmt[kc][:, :, ot * 128:(ot + 1) * 128], ps
                    )

                # xt prefetch at chunk boundaries
                for s_, h_ in XT_PLAN[kc]:
                    issue_xt(s_, h_)

            # ---------------- phase 2: GEMM ------------------------------
            for st in range(N_STRIP):
                issue_xt(st, 0); issue_xt(st, 1)
                accs = [ps_gemm.tile([128, O], f32, tag="acc", name=f"acc{st}_{i}")
                        for i in range(2)]
                early = st < N_EARLY
                act_out = early or _os.environ.get("KV2_OUT") == "act"
                for h in range(2):
                    if act_out:
                        # bias as the zeroing first matmul; chunk-0 o-sliced
                        # so columns start as each ot's wmT copy lands
                        nc.tensor.matmul(accs[h], ones_t, bias_bf,
                                         start=True, stop=False,
                                         skip_group_check=True)
                    for kc in range(N_KC):
                        for j in range(8):
                            tl = (kc % 2) * 8 + j
                            lhs = xt_tiles[(st, kc // 2)][:, tl,
                                                          h * 128:(h + 1) * 128]
                            if early and kc == 0:
                                for ot in range(N_OT):
                                    osl = slice(ot * 128, (ot + 1) * 128)
                                    nc.tensor.matmul(
                                        accs[h][:, osl], lhs,
                                        wmt[kc][:, j, osl],
                                        start=False, stop=False,
                                        skip_group_check=True,
                                    )
                            else:
                                nc.tensor.matmul(
                                    accs[h], lhs, wmt[kc][:, j, :],
                                    start=(not act_out and kc == 0 and j == 0),
                                    stop=(kc == N_KC - 1 and j == 7),
                                    skip_group_check=act_out,
                                )
                for h in range(2):
                    o_t = out_pool.tile([128, O], bf16, tag="o",
                                        name=f"o_{st}_{h}")
                    if act_out:
                        nc.scalar.copy(o_t, accs[h])
                    else:
                        nc.vector.tensor_add(o_t, accs[h], bias_s)
                    nc.sync.dma_start(
                        out=out_d[st * TS + h * 128:st * TS + (h + 1) * 128, :],
                        in_=o_t,
                    )

    nc.compile()
    return nc


def _get_program(mode, const_c=None):
    key = mode
    if key not in _prog_cache:
        _prog_cache[key] = _build_program(mode)
    return _prog_cache[key]


# s-plane order for cols/w packing: cols tile holds [col2, col1, col3, col0]
PLANE_ORDER = (2, 1, 3, 0)


def _k_index():
    """K_IDX[tile, p] = source k for GEMM tile `tile`=kc*8+j, partition p.
    XBAR transpose of wm [128 o, 1024 f] lands f at out (p, j) = (f//8, f%8);
    f = i*256 + kb_local with cols-plane i -> k-offset PLANE_ORDER[i]."""
    tl = np.arange(32)[:, None]          # kc*8 + j
    p = np.arange(128)[None, :]
    kc = tl // 8
    f = (tl % 8) * 128 + p
    i = f // KC_B
    kb_local = f % KC_B
    s = np.asarray(PLANE_ORDER)[i]
    return 4 * (kc * KC_B + kb_local) + s


def _pack_host(x, weight, bias, gumbel_noise):
    import ml_dtypes
    bf16 = ml_dtypes.bfloat16

    x2 = np.asarray(x, np.float32).reshape(T, K).astype(bf16)
    kidx = _k_index().reshape(-1)
    # xt_packed[strip, p, tile, t] = x[strip*TS + t, K_IDX[tile, p]]
    xg = x2[:, kidx].reshape(N_STRIP, TS, 32, 128)
    xt_packed = np.ascontiguousarray(xg.transpose(0, 3, 2, 1)).reshape(
        N_STRIP * 128, 32 * TS
    )

    w = np.asarray(weight, np.float32).astype(bf16)
    b = np.asarray(bias, np.float32).reshape(1, O_FULL)
    g = np.asarray(gumbel_noise, np.float32).reshape(O_FULL, GUM_COLS)
    return xt_packed, w, b, g


def _pack_w_core(w_core):
    # w_packed[(kc, ot), po, s(plane order), kb]
    w6 = w_core.reshape(N_OT, 128, N_KC, KC_B, 4)  # [ot, po, kc, kb, s]
    w6 = w6[..., PLANE_ORDER]
    return np.ascontiguousarray(w6.transpose(2, 0, 1, 4, 3)).reshape(
        16 * 128, 1024
    )


def kernel(x, weight, bias, choice_weights, gumbel_noise):
    from concourse.bass_utils import run_bass_kernel_spmd

    cw = np.asarray(choice_weights, np.float32)
    c0 = float(cw.flat[0])
    is_const = bool((cw == c0).all())
    mode = "const" if is_const else "full"
    nc = _get_program(mode)

    xt_packed, w, b, g = _pack_host(x, weight, bias, gumbel_noise)

    in_maps = []
    for c in range(N_CORES):
        rows = slice(c * O, (c + 1) * O)
        m = {
            "xt": xt_packed,
            "w": _pack_w_core(w[rows]),
            "b": np.ascontiguousarray(b[:, rows]),
            "g": np.ascontiguousarray(g[rows]),
        }
        if mode == "full":
            m["cw"] = np.ascontiguousarray(cw.reshape(O_FULL, GUM_COLS)[rows])
        in_maps.append(m)

    res = run_bass_kernel_spmd(nc, in_maps, list(range(N_CORES)))
    parts = [np.asarray(res.results[c]["out"]).astype(np.float32)
             for c in range(N_CORES)]
    out = np.concatenate(parts, axis=1)  # [T, O_FULL]
    return out.reshape(2, 2048, O_FULL)


# revision 4
# speedup vs baseline: 1.3698x; 1.0074x over previous
"""Trainium2 Bass kernel for Gumbel 2:4-masked Linear (tensor-parallel over out_features).

Forward value (matches reference): mask = PATTERNS[argmax(cw + g, axis=-1)],
out = x @ (W * mask).T + b.  With constant choice_weights the argmax is
shift-invariant, so the mask is PATTERNS[argmax(g)].

v2 design (cost-model driven):
  - bf16 GEMM (x, W, masked W): same PE rate as f32r in the cost model but
    half the DMA traffic and 2-4x DVE rate on mask math. Gumbel stays f32
    (bf16 would flip ~1e-3 of the argmaxes).
  - Mask build per (kc, ot) tile [128 o, 256 blocks]: one max-reduce, one
    batched is_ge (6 planes at once, broadcast max), 5 batched/single bf16
    adds to form the 4 mask columns, one bf16 mul with host-plane-packed W.
  - Reduce+compare engine alternates DVE/Pool per iter to balance; adds+mul
    on DVE; PSUM->SBUF copies on Act (otherwise idle); transposes on PE.
  - Bias folded into each PSUM accumulation chain as a rank-1 matmul
    (ones[1,128].T @ bias[1,512]); Act copies PSUM->SBUF bf16; DMA out bf16.
  - Host pre-packs xT/w so every DMA moves >=512B contiguous runs
    (full 360 GB/s in the model): per-strip xT descriptors are 8 KB.
  - xt strip loads interleaved into phase 1 at chunk boundaries so the GEMM
    can start as soon as wmt[0] lands.
"""

import numpy as np

N_CORES = 8
T = 4096            # tokens
K = 4096            # in_features
O_FULL = 4096
O = O_FULL // N_CORES        # 512 out rows per core
B = K // 4                   # 1024 blocks per out row
GUM_COLS = B * 6             # 6144
N_KC = 4                     # k chunks
KC_B = B // N_KC             # 256 blocks per chunk
N_OT = O // 128              # 4 o-tiles
N_STRIP = 16                 # token strips
TS = T // N_STRIP            # 256 tokens per strip

import os as _os

# iters whose reduce+compare run on Pool (rest on DVE); tuned for balance
_POOL_SETS = {
    "16": list(range(16)),
    "14": [i for i in range(16) if i not in (0, 3)],
    "12": [1, 2, 4, 5, 6, 8, 9, 11, 12, 13, 14, 15],
    "10": [1, 2, 5, 6, 8, 9, 11, 12, 14, 15],
    "8": [1, 3, 5, 7, 8, 10, 12, 14],
    "10c": [0, 1, 2, 5, 6, 8, 9, 11, 12, 14],
    "10d": [0, 1, 2, 3, 5, 6, 8, 9, 12, 14],
    "11b": [0, 1, 2, 3, 5, 6, 8, 9, 11, 12, 14],
    "12b": [0, 1, 2, 3, 4, 5, 6, 8, 9, 11, 12, 14],
}
POOL_ITERS = frozenset(_POOL_SETS[_os.environ.get("KV2_POOL", "10c")])
# strips using bias-matmul zeroing + o-sliced chunk-0 + Act-copy out
N_EARLY = int(_os.environ.get("KV2_EARLY", "0"))
# xt (strip, half) prefetches at each chunk boundary
_XT_PLANS = {
    "A": [[(0, 0), (1, 0)], [(2, 0), (0, 1)], [(1, 1), (3, 0), (2, 1)],
          [(3, 1)]],
    "B": [[(0, 0)], [(1, 0)], [(0, 1), (2, 0)], [(1, 1), (3, 0)]],
    "C": [[(0, 0)], [(1, 0), (2, 0)], [(0, 1), (1, 1)], [(2, 1), (3, 0)]],
    "D": [[(0, 0)], [(1, 0)], [(2, 0)], [(0, 1), (3, 0)]],
    "E": [[(0, 0)], [(1, 0), (2, 0)], [(3, 0), (0, 1), (4, 0)],
          [(1, 1), (5, 0)]],
    "F": [[(0, 0)], [(1, 0), (2, 0)], [(3, 0), (4, 0), (0, 1)],
          [(5, 0), (1, 1), (6, 0), (2, 1)]],
    "G": [[(0, 0), (1, 0)], [(2, 0), (3, 0)], [(4, 0), (0, 1), (5, 0)],
          [(1, 1), (6, 0), (2, 1)]],
}
XT_PLAN = _XT_PLANS[_os.environ.get("KV2_XT", "C")]

_prog_cache = {}


def _build_program(mode):
    """mode: 'const' (choice folded away) or 'full' (adds choice tensor)."""
    import concourse.bacc as bacc
    import concourse.bass as bass
    import concourse.mybir as mybir
    import concourse.tile as tile
    from concourse.masks import make_identity

    f32 = mybir.dt.float32
    bf16 = mybir.dt.bfloat16
    Alu = mybir.AluOpType

    nc = bacc.Bacc(trn_type="TRN2")
    # xt packed: [strip, kbp, (kc, s, kbt), t] -> [16*128, 32*256] bf16
    xt_d = nc.declare_dram_parameter("xt", [N_STRIP * 128, 32 * TS], bf16,
                                     isOutput=False)
    # w packed: [(kc, ot), po, (s, kb)] -> [16*128, 1024] bf16
    w_d = nc.declare_dram_parameter("w", [16 * 128, 1024], bf16, isOutput=False)
    b_d = nc.declare_dram_parameter("b", [1, O], f32, isOutput=False)
    g_d = nc.declare_dram_parameter("g", [O, GUM_COLS], f32, isOutput=False)
    if mode == "full":
        cw_d = nc.declare_dram_parameter("cw", [O, GUM_COLS], f32, isOutput=False)
    out_d = nc.declare_dram_parameter("out", [T, O], bf16, isOutput=True)

    with tile.TileContext(nc) as tc:
        with (
            tc.tile_pool(name="singles", bufs=1) as singles,
            tc.tile_pool(name="wmt", bufs=N_KC) as wmt_pool,
            tc.tile_pool(name="gum", bufs=4) as gum_pool,
            tc.tile_pool(name="wtile", bufs=4) as w_pool,
            tc.tile_pool(name="msk", bufs=3) as msk,
            tc.tile_pool(name="xth", bufs=8) as xt_pool,
            tc.tile_pool(name="outs", bufs=3) as out_pool,
            tc.tile_pool(name="ps_xpose", bufs=int(_os.environ.get("KV2_XPB", "1")),
                         space="PSUM") as ps_xpose,
            tc.tile_pool(name="ps_gemm", bufs=int(_os.environ.get("KV2_ACCB", "7")),
                         space="PSUM") as ps_gemm,
        ):
            ident_f32 = singles.tile([128, 128], f32)
            make_identity(nc, ident_f32)
            ident = singles.tile([128, 128], bf16, name="ident_bf")
            nc.vector.tensor_copy(ident, ident_f32)
            bias_s = singles.tile([128, O], f32, name="bias_s")
            nc.gpsimd.dma_start(
                out=bias_s,
                in_=bass.AP(tensor=b_d, offset=0, ap=[[0, 128], [1, O]]),
            )
            bias_bf = singles.tile([1, O], bf16, name="bias_bf")
            nc.vector.tensor_copy(bias_bf, bias_s[0:1, :])
            ones_t = singles.tile([1, 128], bf16, name="ones")
            nc.vector.memset(ones_t, 1.0)

            # resident transposed masked weight, one tile per k chunk:
            # wmt[kc][kbp, j=(s,kbt), o]
            wmt = [
                wmt_pool.tile([128, 8, O], bf16, name=f"wmt{i}", tag=f"wmt{i}",
                              bufs=1)
                for i in range(N_KC)
            ]

            xt_tiles = {}  # (strip, half) -> tile

            def issue_xt(s, h):
                if (s, h) in xt_tiles:
                    return
                t_ = xt_pool.tile([128, 16, TS], bf16, tag="xth")
                nc.sync.dma_start(
                    out=t_,
                    in_=xt_d[s * 128:(s + 1) * 128,
                             h * 16 * TS:(h + 1) * 16 * TS],
                )
                xt_tiles[(s, h)] = t_

            # ---------------- phase 1: masks + masked W^T -----------------
            for kc in range(N_KC):
                g_tiles = [None] * N_OT
                w_tiles = [None] * N_OT

                def _load_g(ot, kc=kc, g_tiles=g_tiles):
                    rows = slice(ot * 128, (ot + 1) * 128)
                    t_ = gum_pool.tile([128, KC_B * 6], f32, tag="gum",
                                       name=f"g_{kc}_{ot}")
                    nc.sync.dma_start(
                        out=t_, in_=g_d[rows, kc * KC_B * 6:(kc + 1) * KC_B * 6]
                    )
                    g_tiles[ot] = t_

                def _load_w(ot, kc=kc, w_tiles=w_tiles):
                    t_ = w_pool.tile([128, 1024], bf16, tag="w",
                                     name=f"w_{kc}_{ot}")
                    nc.sync.dma_start(
                        out=t_, in_=w_d[(kc * N_OT + ot) * 128:
                                        (kc * N_OT + ot + 1) * 128, :]
                    )
                    w_tiles[ot] = t_

                if kc == 0:
                    # cold start: g00 first (mask pipe), then all w (muls),
                    # then remaining g
                    _load_g(0)
                    for ot in range(N_OT):
                        _load_w(ot)
                    for ot in range(1, N_OT):
                        _load_g(ot)
                else:
                    for ot in range(N_OT):
                        _load_g(ot)
                    for ot in range(N_OT):
                        _load_w(ot)
                for ot in range(N_OT):
                    it = kc * N_OT + ot
                    # Pool is add/sub/mult-only on TRN2; reduce+compare are
                    # DVE-only. Pool takes adds+mul for POOL_ITERS.
                    eng = nc.gpsimd if it in POOL_ITERS else nc.vector
                    rows = slice(ot * 128, (ot + 1) * 128)
                    g_t = g_tiles[ot]
                    if mode == "full":
                        cw_t = gum_pool.tile([128, KC_B * 6], f32, tag="cw")
                        nc.sync.dma_start(
                            out=cw_t,
                            in_=cw_d[rows, kc * KC_B * 6:(kc + 1) * KC_B * 6],
                        )
                        nc.vector.tensor_add(g_t, g_t, cw_t)
                    g3 = g_t.rearrange("p (b s) -> p b s", s=6)

                    m = msk.tile([128, KC_B], f32, tag="m")
                    nc.vector.tensor_reduce(m, g3, axis=mybir.AxisListType.X,
                                            op=Alu.max)

                    # one-hot planes e[s][kb] (multi-hot only on exact ties)
                    e_t = msk.tile([128, 6, KC_B], bf16, tag="e")
                    gsb = g_t.rearrange("p (b s) -> p s b", s=6)
                    m_b = m.unsqueeze(1).broadcast_to([128, 6, KC_B])
                    nc.vector.tensor_tensor(e_t, gsb, m_b, op=Alu.is_ge)

                    # cols storage order: [col2, col1, col3, col0]
                    # col0=e3+e4+e5  col1=e1+e2+e5  col2=e0+e2+e4  col3=e0+e1+e3
                    ev = e_t  # [128, 6, 256]
                    s2 = msk.tile([128, 2, KC_B], bf16, tag="s2")
                    # s2 = [e0+e1, e4+e5]  (cheap on DVE; Pool's 0.42 eff loses)
                    nc.vector.tensor_add(s2, ev[:, 0::4, :], ev[:, 1::4, :])
                    t2 = msk.tile([128, 2, KC_B], bf16, tag="t2")
                    # t2 = [e0+e2, e1+e2]
                    eng.tensor_add(t2, ev[:, 0:2, :],
                                   ev[:, 2:3, :].broadcast_to([128, 2, KC_B]))
                    cols = msk.tile([128, 4, KC_B], bf16, tag="cols")
                    # [col2, col1] = t2 + [e4, e5]
                    eng.tensor_add(cols[:, 0:2, :], t2, ev[:, 4:6, :])
                    # [col3, col0] = s2 + e3
                    eng.tensor_add(cols[:, 2:4, :], s2,
                                   ev[:, 3:4, :].broadcast_to([128, 2, KC_B]))

                    w_t = w_tiles[ot]
                    # wm[o, s, kb] = w[o, s, kb] * col_s[o, kb]
                    # w packed with s-plane order (2, 1, 3, 0) to match cols
                    wm = w_pool.tile([128, 1024], bf16, tag="wm")
                    eng.tensor_mul(
                        wm, w_t, cols.rearrange("p s b -> p (s b)")
                    )

                    # transpose 8 subtiles [o, f] -> [f, o]; psum tile j holds
                    # f-range [j*128, (j+1)*128) (same map as host K_IDX)
                    ps = ps_xpose.tile([128, 8, 128], bf16, tag="psx")
                    wmv = wm.rearrange("p (j b) -> p j b", j=8)
                    for j in range(8):
                        nc.tensor.transpose(ps[:, j, :], wmv[:, j, :], ident)
                    nc.scalar.copy(
                        wmt[kc][:, :, ot * 128:(ot + 1) * 128], ps
                    )

                # xt prefetch at chunk boundaries
                for s_, h_ in XT_PLAN[kc]:
                    issue_xt(s_, h_)

            # ---------------- phase 2: GEMM ------------------------------
            for st in range(N_STRIP):
                issue_xt(st, 0); issue_xt(st, 1)
                accs = [ps_gemm.tile([128, O], f32, tag="acc", name=f"acc{st}_{i}")
                        for i in range(2)]
                early = st < N_EARLY and _os.environ.get("KV2_OSL", "1") == "1"
                act_out = (st < N_EARLY) or _os.environ.get("KV2_OUT") == "act"
                for h in range(2):
                    if act_out:
                        # bias as the zeroing first matmul; chunk-0 o-sliced
                        # so columns start as each ot's wmT copy lands
                        nc.tensor.matmul(accs[h], ones_t, bias_bf,
                                         start=True, stop=False,
                                         skip_group_check=True)
                    for kc in range(N_KC):
                        for j in range(8):
                            tl = (kc % 2) * 8 + j
                            lhs = xt_tiles[(st, kc // 2)][:, tl,
                                                          h * 128:(h + 1) * 128]
                            if early and kc == 0:
                                for ot in range(N_OT):
                                    osl = slice(ot * 128, (ot + 1) * 128)
                                    nc.tensor.matmul(
                                        accs[h][:, osl], lhs,
                                        wmt[kc][:, j, osl],
                                        start=False, stop=False,
                                        skip_group_check=True,
                                    )
                            else:
                                nc.tensor.matmul(
                                    accs[h], lhs, wmt[kc][:, j, :],
                                    start=(not act_out and kc == 0 and j == 0),
                                    stop=(kc == N_KC - 1 and j == 7),
                                    skip_group_check=act_out,
                                )
                for h in range(2):
                    o_t = out_pool.tile([128, O], bf16, tag="o",
                                        name=f"o_{st}_{h}")
                    if act_out:
                        nc.scalar.copy(o_t, accs[h])
                    else:
                        nc.vector.tensor_add(o_t, accs[h], bias_s)
                    nc.sync.dma_start(
                        out=out_d[st * TS + h * 128:st * TS + (h + 1) * 128, :],
                        in_=o_t,
                    )

    nc.compile()
    return nc


def _get_program(mode, const_c=None):
    key = mode
    if key not in _prog_cache:
        _prog_cache[key] = _build_program(mode)
    return _prog_cache[key]


# s-plane order for cols/w packing: cols tile holds [col2, col1, col3, col0]
PLANE_ORDER = (2, 1, 3, 0)


def _k_index():
    """K_IDX[tile, p] = source k for GEMM tile `tile`=kc*8+j, partition p.
    XBAR transpose of wm [128 o, 1024 f] lands f at out (p, j) = (f//8, f%8);
    f = i*256 + kb_local with cols-plane i -> k-offset PLANE_ORDER[i]."""
    tl = np.arange(32)[:, None]          # kc*8 + j
    p = np.arange(128)[None, :]
    kc = tl // 8
    f = (tl % 8) * 128 + p
    i = f // KC_B
    kb_local = f % KC_B
    s = np.asarray(PLANE_ORDER)[i]
    return 4 * (kc * KC_B + kb_local) + s


def _pack_host(x, weight, bias, gumbel_noise):
    import ml_dtypes
    bf16 = ml_dtypes.bfloat16

    x2 = np.asarray(x, np.float32).reshape(T, K).astype(bf16)
    kidx = _k_index().reshape(-1)
    # xt_packed[strip, p, tile, t] = x[strip*TS + t, K_IDX[tile, p]]
    xg = x2[:, kidx].reshape(N_STRIP, TS, 32, 128)
    xt_packed = np.ascontiguousarray(xg.transpose(0, 3, 2, 1)).reshape(
        N_STRIP * 128, 32 * TS
    )

    w = np.asarray(weight, np.float32).astype(bf16)
    b = np.asarray(bias, np.float32).reshape(1, O_FULL)
    g = np.asarray(gumbel_noise, np.float32).reshape(O_FULL, GUM_COLS)
    return xt_packed, w, b, g


def _pack_w_core(w_core):
    # w_packed[(kc, ot), po, s(plane order), kb]
    w6 = w_core.reshape(N_OT, 128, N_KC, KC_B, 4)  # [ot, po, kc, kb, s]
    w6 = w6[..., PLANE_ORDER]
    return np.ascontiguousarray(w6.transpose(2, 0, 1, 4, 3)).reshape(
        16 * 128, 1024
    )


def kernel(x, weight, bias, choice_weights, gumbel_noise):
    from concourse.bass_utils import run_bass_kernel_spmd

    cw = np.asarray(choice_weights, np.float32)
    c0 = float(cw.flat[0])
    is_const = bool((cw == c0).all())
    mode = "const" if is_const else "full"
    nc = _get_program(mode)

    xt_packed, w, b, g = _pack_host(x, weight, bias, gumbel_noise)

    in_maps = []
    for c in range(N_CORES):
        rows = slice(c * O, (c + 1) * O)
        m = {
            "xt": xt_packed,
            "w": _pack_w_core(w[rows]),
            "b": np.ascontiguousarray(b[:, rows]),
            "g": np.ascontiguousarray(g[rows]),
        }
        if mode == "full":
            m["cw"] = np.ascontiguousarray(cw.reshape(O_FULL, GUM_COLS)[rows])
        in_maps.append(m)

    res = run_bass_kernel_spmd(nc, in_maps, list(range(N_CORES)))
    parts = [np.asarray(res.results[c]["out"]).astype(np.float32)
             for c in range(N_CORES)]
    out = np.concatenate(parts, axis=1)  # [T, O_FULL]
    return out.reshape(2, 2048, O_FULL)
